# revision 50
# baseline (speedup 1.0000x reference)
"""LlamaAttention forward on 8 Trainium2 NeuronCores (Bass/Tile).

Sharding: 8 cores = 2 batches x 4 head-groups (4 heads each).
Per core: QKV projections (column-parallel over heads), RoPE, causal eager
attention for its 4 heads, and a row-parallel partial O-projection; the host
sums the 4 partials per batch.

Schedule (causal fast path):
  phase 1:  [V st0..7][Q h0][K h0][att span0 h0][V st8..11][Q h1][K h1]
            [att span0 h1][V 12..15][Q h2][K h2][att s0 h2][Q h3][K h3]
            [att s0 h3] -- span-0 attention (ACT-bound) hides behind the
            projection matmuls; RoPE chains are deferred one unit so the
            rot-matmul never stalls the PE behind the ACT psum-copy.
  phase 2:  spans 1..3; the O-projection of span s-1 is injected *inside*
            the attention kc-loops (4 x 4-matmul bundles per head) to keep
            the PE fed while ACT computes exp; remaining O tiles drain at
            the end with per-ospan output stores.

Everything is bf16 (same PE rate as fp32r, half the DMA/SBUF) with fp32
PSUM accumulation; Q/K/V stay SBUF-resident; blocks above the causal
diagonal are skipped, sub-diagonal blocks skip the mask multiply, and only
the diagonal band of exp(mask) is shipped.  Softmax denominators come from
a DVE bf16 accumulation + one gpsimd partition_all_reduce per head-span
(no ones-matmuls on the PE).  exp runs on unshifted scores (they are O(1);
the 1/sqrt(hd) scale is folded into the q-side RoPE tables).
"""

import math
import sys
from contextlib import ExitStack

import numpy as np

try:
    import concourse.bass as bass
except ImportError:  # pragma: no cover - fallback when PYTHONPATH is not set
    sys.path.insert(0, "/opt/trn_rl_repo")
    import concourse.bass as bass

import concourse.bacc as bacc
import concourse.tile as tile
from concourse import mybir
from concourse import bass_isa
from concourse.bass import ds, ts
from concourse.bass_utils import run_bass_kernel_spmd

import ml_dtypes

# Problem constants (hardcoded per spec)
B, S, H = 2, 2048, 2048
N_HEADS, HD = 16, 128
ROPE_BASE = 10000.0
N_CORES = 8
GROUPS = 4          # head-groups (tensor parallel)
HPG = N_HEADS // GROUPS  # heads per group = 4
D = HPG * HD        # per-core projected dim = 512
P = 128
HC = H // P         # contraction chunks for projections = 16
STILES = S // P     # s tiles = 16
NSPAN = 4           # 512-wide spans of S
SPAN = S // NSPAN   # 512
KC = S // P         # k chunks = 16

F32 = mybir.dt.float32
BF16 = mybir.dt.bfloat16
MUL = mybir.AluOpType.mult
ADD = mybir.AluOpType.add
EXP = mybir.ActivationFunctionType.Exp
COPY = mybir.ActivationFunctionType.Copy
RADD = bass_isa.ReduceOp.add

_CACHED_NC = None          # causal-structure kernel (fast path)
_CACHED_NC_GENERIC = None  # arbitrary-mask fallback


def build_nc(causal=True):
    nc = bacc.Bacc("TRN2", target_bir_lowering=False)

    # x pre-tiled on host: [p, st, hc, si] so both the V (st-chunk) and the
    # Q/K (span) access patterns stream contiguous >=512B runs.
    xt4 = nc.declare_dram_parameter("xt4", [P, STILES, HC, P], BF16, isOutput=False)
    wq4 = nc.declare_dram_parameter("wq4", [P, HPG, HC, HD], BF16, isOutput=False)
    wk4 = nc.declare_dram_parameter("wk4", [P, HPG, HC, HD], BF16, isOutput=False)
    wv2 = nc.declare_dram_parameter("wv2", [P, HC, D], BF16, isOutput=False)
    wo2 = nc.declare_dram_parameter("wo2", [P, HPG, H], BF16, isOutput=False)
    if causal:
        emd = nc.declare_dram_parameter("emd", [P, KC, SPAN], BF16, isOutput=False)
    else:
        emd = nc.declare_dram_parameter("emd", [P, KC, S], BF16, isOutput=False)
    # sinq/sink arrive sign-folded (rows 0..63 negated) so rotate_half is a
    # pure partition swap done by two SBUF->SBUF DMAs -- no PE matmul needed
    cosq = nc.declare_dram_parameter("cosq", [HD, S], BF16, isOutput=False)
    sinq = nc.declare_dram_parameter("sinq", [HD, S], BF16, isOutput=False)
    cosk = nc.declare_dram_parameter("cosk", [HD, S], BF16, isOutput=False)
    sink = nc.declare_dram_parameter("sink", [HD, S], BF16, isOutput=False)
    out = nc.declare_dram_parameter("out", [S, H], F32, isOutput=True)

    def n_kc(span):
        """number of k-chunks contributing to this q-span"""
        return 4 * span + 4 if causal else KC

    def needs_mask(span, kc):
        return kc >= 4 * span if causal else True

    with tile.TileContext(nc) as tc, ExitStack() as ctx_all:
        # long-lived tensors
        kv_pool = ctx_all.enter_context(tc.tile_pool(name="kv", bufs=1))
        qt_sb = kv_pool.tile([P, HPG, S], BF16)
        kt_sb = kv_pool.tile([P, HPG, S], BF16)
        v_sb = kv_pool.tile([P, KC, D], BF16)
        # one attnT tile+pool per head: coarse-grained RAW tracking would
        # otherwise stall O-projection matmuls on unrelated heads' normalizes
        attnT = []
        for h in range(HPG):
            p_h = ctx_all.enter_context(tc.tile_pool(name=f"attnT{h}", bufs=1))
            attnT.append(p_h.tile([P, S], BF16, name=f"attnT_{h}"))
        em_sb = None
        if causal:
            em_sb = kv_pool.tile([P, KC, SPAN], BF16, name="em_sb")

        # attention working pools (used from phase 1 on for span 0)
        et_pool = ctx_all.enter_context(tc.tile_pool(name="et", bufs=2))
        pt_pool = ctx_all.enter_context(tc.tile_pool(name="pt", bufs=2))
        acc_pool = ctx_all.enter_context(tc.tile_pool(name="acc", bufs=2))
        red_pool = ctx_all.enter_context(tc.tile_pool(name="red", bufs=1))
        rc_pool = ctx_all.enter_context(tc.tile_pool(name="rc", bufs=1))
        psum_at = ctx_all.enter_context(
            tc.tile_pool(name="psum_at", bufs=2, space="PSUM")
        )
        # shared space: holds wv during phase 1, recycled for wo afterwards
        wvwo_pool = ctx_all.enter_context(tc.tile_pool(name="wvwo", bufs=1))

        # O-projection task machinery (tasks injected into attention loops)
        o_state = {"queue": [], "emit": None}

        def pump_o():
            if o_state["queue"] and o_state["emit"] is not None:
                o_state["emit"](*o_state["queue"].pop(0))

        def attention_unit(span, head, inject, sc_pool, paired=False):
            nk = n_kc(span)
            marks = {((i + 1) * nk) // 5 for i in range(4)} if inject else ()
            qh = qt_sb[:, head, ts(span, SPAN)]
            aps = psum_at.tile([P, SPAN], F32, tag="atps")
            acc = acc_pool.tile([P, SPAN], BF16, tag="acc")
            step = 2 if paired else 1
            for kc0 in range(0, nk, step):
                kcs = range(kc0, kc0 + step)
                # scores for `step` k-blocks share one psum tile so exp's
                # per-instruction access latency is amortized
                sc = sc_pool.tile([P, step, SPAN], F32, tag="scps")
                for i, kc in enumerate(kcs):
                    nc.tensor.matmul(
                        sc[:, i],
                        lhsT=kt_sb[:, head, ts(kc, P)],
                        rhs=qh,
                        start=True,
                        stop=True,
                    )
                et = et_pool.tile([P, step, SPAN], BF16, tag="et")
                nc.scalar.activation(et[:], sc[:], EXP)
                if needs_mask(span, kc0):
                    if causal:
                        emb = em_sb[:, ds(kc0, step)]
                    else:
                        emb = em_sb[:, ds(kc0, step), ts(span, SPAN)]
                    pt = pt_pool.tile([P, step, SPAN], BF16, tag="pt")
                    nc.vector.tensor_tensor(pt[:], et[:], emb, MUL)
                    p_use = pt
                else:
                    p_use = et
                for i, kc in enumerate(kcs):
                    nc.tensor.matmul(
                        aps[:],
                        lhsT=v_sb[:, kc, ts(head, HD)],
                        rhs=p_use[:, i],
                        start=(kc == 0),
                        stop=(kc == nk - 1),
                    )
                    if kc == 0:
                        nc.vector.tensor_copy(acc[:], p_use[:, i])
                    else:
                        nc.vector.tensor_tensor(acc[:], acc[:], p_use[:, i], ADD)
                    if kc in marks:
                        pump_o()
            red = red_pool.tile([P, SPAN], F32, tag="red")
            nc.gpsimd.partition_all_reduce(
                red[:], acc[:], channels=P, reduce_op=RADD
            )
            rcb = rc_pool.tile([P, SPAN], F32, tag="rcb")
            nc.vector.reciprocal(rcb[:], red[:])
            nc.vector.tensor_tensor(
                attnT[head][:, ts(span, SPAN)], aps[:], rcb[:], MUL
            )

        # ================= Phase 1: projections + RoPE (+ span-0 attn) =====
        with ExitStack() as ctx_p1:
            xt_pool = ctx_p1.enter_context(tc.tile_pool(name="xt", bufs=1))
            psum_v = ctx_p1.enter_context(
                tc.tile_pool(name="psum_v", bufs=2, space="PSUM")
            )
            tab_pool = ctx_p1.enter_context(tc.tile_pool(name="tab", bufs=1))
            wp_pool = ctx_p1.enter_context(tc.tile_pool(name="wp", bufs=2))
            rawp = ctx_p1.enter_context(tc.tile_pool(name="rawp", bufs=3))
            rswp = ctx_p1.enter_context(tc.tile_pool(name="rswp", bufs=3))
            t1p = ctx_p1.enter_context(tc.tile_pool(name="t1p", bufs=2))
            t2p = ctx_p1.enter_context(tc.tile_pool(name="t2p", bufs=2))
            psum_sc = ctx_p1.enter_context(
                tc.tile_pool(name="psum_sc", bufs=4, space="PSUM")
            )

            xt_sb = xt_pool.tile([P, STILES, HC, P], BF16)
            wv_sb = wvwo_pool.tile([P, HC, D], BF16, tag="wvwo", name="wv_sb")

            # input DMAs, interleaved so V st0 can start ~3us in
            for g in range(4):
                nc.sync.dma_start(xt_sb[:, 0, ts(g, 4)], xt4[:, 0, ts(g, 4)])
                nc.sync.dma_start(wv_sb[:, ts(g, 4)], wv2[:, ts(g, 4)])
            for st in range(1, 8):
                nc.sync.dma_start(xt_sb[:, st], xt4[:, st])
            cq = tab_pool.tile([HD, S], BF16)
            sq = tab_pool.tile([HD, S], BF16)
            ck = tab_pool.tile([HD, S], BF16)
            sk = tab_pool.tile([HD, S], BF16)
            nc.sync.dma_start(cq[:], cosq[:])
            nc.sync.dma_start(sq[:], sinq[:])
            for st in range(8, STILES):
                nc.sync.dma_start(xt_sb[:, st], xt4[:, st])
            nc.sync.dma_start(ck[:], cosk[:])
            nc.sync.dma_start(sk[:], sink[:])
            if causal:
                nc.sync.dma_start(em_sb[:], emd[:])

            def vunit(st):
                ps = psum_v.tile([P, D], F32, tag="vps")
                for hc in range(HC):
                    nc.tensor.matmul(
                        ps[:],
                        lhsT=xt_sb[:, st, hc],
                        rhs=wv_sb[:, hc],
                        start=(hc == 0),
                        stop=(hc == HC - 1),
                    )
                nc.scalar.activation(v_sb[:, st], ps[:], COPY)

            # deferred rope: (psum, cos_ap, sin_ap, dest_ap)
            pending = [None]

            def emit_rope():
                ps, tabc, tabs, dest_ap = pending[0]
                pending[0] = None
                raw = rawp.tile([P, SPAN], BF16, tag="raw")
                nc.scalar.activation(raw[:], ps[:], COPY)
                # rotate_half = partition-swap (sign lives in the sin tables)
                rsw = rswp.tile([P, SPAN], BF16, tag="rsw")
                nc.sync.dma_start(rsw[0 : HD // 2], raw[HD // 2 : HD])
                nc.sync.dma_start(rsw[HD // 2 : HD], raw[0 : HD // 2])
                t1 = t1p.tile([P, SPAN], BF16, tag="t1")
                nc.vector.tensor_tensor(t1[:], raw[:], tabc, MUL)
                t2 = t2p.tile([P, SPAN], BF16, tag="t2")
                nc.vector.tensor_tensor(t2[:], rsw[:], tabs, MUL)
                nc.vector.tensor_tensor(dest_ap, t1[:], t2[:], ADD)

            def qkblock(head, w4, tabc, tabs, dest, mid_cb=None):
                wp = wp_pool.tile([P, HC, HD], BF16, tag="wp")
                nc.sync.dma_start(wp[:], w4[:, head])
                for span in range(NSPAN):
                    ps = psum_sc.tile([P, SPAN], F32, tag="scps")
                    for hc in range(HC):
                        nc.tensor.matmul(
                            ps[:],
                            lhsT=wp[:, hc],
                            rhs=xt_sb[:, ts(span, 4), hc, :],
                            start=(hc == 0),
                            stop=(hc == HC - 1),
                        )
                    if pending[0] is not None:
                        emit_rope()
                    pending[0] = (
                        ps,
                        tabc[:, ts(span, SPAN)],
                        tabs[:, ts(span, SPAN)],
                        dest[:, head, ts(span, SPAN)],
                    )
                    if span == 2 and mid_cb is not None:
                        # span-0 rope finished during span 1/2: the callback's
                        # attention unit overlaps the span-3 matmuls
                        mid_cb()

            v_rounds = [range(0, 8), range(8, 12), range(12, 16), range(0, 0)]
            wo_sb = None
            for head in range(HPG):
                for st in v_rounds[head]:
                    vunit(st)
                if head == HPG - 1:
                    # all V units emitted: recycle wv's space for wo so
                    # span-1's O-projection never waits on its load
                    wo_sb = wvwo_pool.tile(
                        [P, HPG, H], BF16, tag="wvwo", name="wo_sb"
                    )
                    nc.sync.dma_start(wo_sb[:], wo2[:])
                qkblock(head, wq4, cq, sq, qt_sb)
                att0 = (
                    (
                        lambda h: lambda: attention_unit(
                            0, h, inject=False, sc_pool=psum_sc
                        )
                    )(head)
                    if causal
                    else None
                )
                if head == HPG - 1:
                    # last head: run span-0 attention inside the K block so
                    # its ACT chain hides behind the span-3 matmuls
                    qkblock(head, wk4, ck, sk, kt_sb, mid_cb=att0)
                else:
                    qkblock(head, wk4, ck, sk, kt_sb)
                    if att0 is not None:
                        att0()
            if pending[0] is not None:
                emit_rope()

        # ================= Phase 2 + 3 =================
        with ExitStack() as ctx_p23:
            psum_sc2 = ctx_p23.enter_context(
                tc.tile_pool(name="psum_sc2", bufs=2, space="PSUM")
            )
            if not causal:
                emf_pool = ctx_p23.enter_context(tc.tile_pool(name="emf", bufs=1))
                em_sb = emf_pool.tile([P, KC, S], BF16, name="em_sb")
                for g in range(4):
                    nc.sync.dma_start(em_sb[:, ts(g, 4)], emd[:, ts(g, 4)])
            o_pool = ctx_p23.enter_context(tc.tile_pool(name="o", bufs=6))
            psum_o = ctx_p23.enter_context(
                tc.tile_pool(name="psum_o", bufs=2, space="PSUM")
            )

            o_flip = [0]

            def emit_o_mms(ps, st, ospan, dcs, start, stop):
                for i, dc in enumerate(dcs):
                    nc.tensor.matmul(
                        ps[:],
                        lhsT=attnT[dc][:, ts(st, P)],
                        rhs=wo_sb[:, dc, ts(ospan, SPAN)],
                        start=start and (i == 0),
                        stop=stop and (i == len(dcs) - 1),
                    )

            def emit_o_store(ps, st, ospan):
                o_sb = o_pool.tile([P, SPAN], F32, tag="o")
                if o_flip[0] % 2 == 0:
                    nc.vector.tensor_copy(o_sb[:], ps[:])
                else:
                    nc.scalar.activation(o_sb[:], ps[:], COPY)
                o_flip[0] += 1
                nc.sync.dma_start(out[ts(st, P), ts(ospan, SPAN)], o_sb[:])

            o_pools = [psum_o, psum_at]

            def emit_o(st, ospan):
                pool = o_pools[o_flip[0] % len(o_pools)]
                ps = pool.tile(
                    [P, SPAN], F32, tag="ops" if pool is psum_o else "atps"
                )
                emit_o_mms(ps, st, ospan, range(HPG), True, True)
                emit_o_store(ps, st, ospan)

            o_state["emit"] = emit_o

            first_span = 1 if causal else 0
            if causal:
                o_state["queue"] = [
                    (st, osp) for st in range(4) for osp in range(NSPAN)
                ]
            for span in range(first_span, NSPAN):
                for head in range(HPG):
                    attention_unit(
                        span, head, inject=True, sc_pool=psum_sc2, paired=True
                    )
                o_state["queue"].extend(
                    (4 * span + i, osp) for i in range(4) for osp in range(NSPAN)
                )
            # drain prologue: open 4 tiles with their dc 0..2 accumulated
            # (independent of the last head's normalize), then finish
            pre = []
            for _ in range(min(4, len(o_state["queue"]))):
                st, osp = o_state["queue"].pop(0)
                pool = o_pools[o_flip[0] % len(o_pools)]
                ps = pool.tile(
                    [P, SPAN], F32, tag="ops" if pool is psum_o else "atps"
                )
                o_flip[0] += 1
                emit_o_mms(ps, st, osp, range(HPG - 1), True, False)
                pre.append((ps, st, osp))
            for ps, st, osp in pre:
                emit_o_mms(ps, st, osp, [HPG - 1], False, True)
                emit_o_store(ps, st, osp)
            while o_state["queue"]:
                pump_o()

    nc.compile()
    return nc


def _rope_tables(position_ids_b):
    """cos/sin tables [S, HD], matching the reference computation."""
    inv_freq = (
        1.0 / (ROPE_BASE ** (np.arange(0, HD, 2, dtype=np.float32) / HD))
    ).astype(np.float32)
    t = np.arange(S, dtype=np.float32)
    freqs = np.outer(t, inv_freq).astype(np.float32)  # [S, HD/2]
    emb = np.concatenate([freqs, freqs], axis=-1)  # [S, HD]
    cos = np.cos(emb).astype(np.float32)
    sin = np.sin(emb).astype(np.float32)
    pos = np.asarray(position_ids_b).astype(np.int64)
    return cos[pos], sin[pos]  # [S, HD] each


def _causal_band(em):
    """Extract the diagonal band [p, 4j+c, q] of exp(mask).T blocks."""
    band = np.empty((P, KC, SPAN), dtype=ml_dtypes.bfloat16)
    for j in range(NSPAN):
        sub = em[512 * j : 512 * j + 512, 512 * j : 512 * j + 512]
        band[:, 4 * j : 4 * j + 4, :] = sub.reshape(4, 128, SPAN).transpose(1, 0, 2)
    return band


def _is_causal_structured(em):
    """True if blocks left of the diagonal band are all-1 and blocks right
    of it are all-0 (band content itself is shipped verbatim)."""
    for j in range(NSPAN):
        qs = slice(512 * j, 512 * j + 512)
        if not np.all(em[: 512 * j, qs] == 1.0):
            return False
        if not np.all(em[512 * j + 512 :, qs] == 0.0):
            return False
    return True


def _make_in_maps(hidden_states, attention_mask, position_ids, Wq, Wk, Wv, Wo):
    hidden_states = np.asarray(hidden_states, dtype=np.float32)
    attention_mask = np.asarray(attention_mask, dtype=np.float32)
    Wq = np.asarray(Wq, dtype=np.float32)
    Wk = np.asarray(Wk, dtype=np.float32)
    Wv = np.asarray(Wv, dtype=np.float32)
    Wo = np.asarray(Wo, dtype=np.float32)
    bf = ml_dtypes.bfloat16

    # rotate_half on-device is a partition swap; the sign of the -x2 half is
    # folded into the sin tables (rows 0..63 negated)
    sin_sign = np.ones((HD, 1), dtype=np.float32)
    sin_sign[: HD // 2] = -1.0

    scale = np.float32(1.0 / np.sqrt(HD))

    # per-group weight tensors (shared across batches)
    gw = []
    for g in range(GROUPS):
        dsl = slice(g * D, (g + 1) * D)
        wq_g = np.empty((P, HPG, HC, HD), dtype=bf)
        wk_g = np.empty((P, HPG, HC, HD), dtype=bf)
        for h in range(HPG):
            rows = slice(g * D + h * HD, g * D + h * HD + HD)
            # W rows [HD, H] -> transpose [H, HD] -> [hc, p, hd] -> [p, hc, hd]
            wq_g[:, h] = (
                Wq[rows, :].T.reshape(HC, P, HD).transpose(1, 0, 2).astype(bf)
            )
            wk_g[:, h] = (
                Wk[rows, :].T.reshape(HC, P, HD).transpose(1, 0, 2).astype(bf)
            )
        wv_g = (
            Wv[dsl, :].T.reshape(HC, P, D).transpose(1, 0, 2).astype(bf)
        )  # [p, hc, d]
        wo_g = (
            Wo[:, dsl].T.reshape(HPG, P, H).transpose(1, 0, 2).astype(bf)
        )  # [p, dc, o]
        gw.append((wq_g, wk_g, wv_g, wo_g))

    causal_all = True
    per_batch = []
    for b in range(B):
        xT = hidden_states[b].T  # [H, S]
        x4 = (
            xT.reshape(HC, P, STILES, P).transpose(1, 2, 0, 3).astype(bf)
        )  # [p, st, hc, si]
        mask_b = attention_mask[b, 0]  # [S, S] additive
        with np.errstate(over="ignore", under="ignore"):
            em = np.exp(mask_b.T, dtype=np.float32)  # [k, q]
        causal_b = _is_causal_structured(em)
        causal_all = causal_all and causal_b
        cos_b, sin_b = _rope_tables(position_ids[b])  # [S, HD]
        cosT = cos_b.T  # [HD, S]
        sinT = sin_b.T * sin_sign
        tables = {
            "cosq": (cosT * scale).astype(bf),
            "sinq": (sinT * scale).astype(bf),
            "cosk": cosT.astype(bf),
            "sink": sinT.astype(bf),
        }
        per_batch.append((x4, em, tables))

    in_maps = []
    for b in range(B):
        x4, em, tables = per_batch[b]
        if causal_all:
            em_t = _causal_band(em)
        else:
            em_t = (
                em.astype(bf).reshape(KC, P, S).transpose(1, 0, 2)
            )  # [p, kc, q]
        for g in range(GROUPS):
            wq_g, wk_g, wv_g, wo_g = gw[g]
            in_maps.append(
                {
                    "xt4": x4,
                    "wq4": wq_g,
                    "wk4": wk_g,
                    "wv2": wv_g,
                    "wo2": wo_g,
                    "emd": np.ascontiguousarray(em_t),
                    **tables,
                }
            )
    return in_maps, causal_all


def kernel(hidden_states, attention_mask, position_ids, Wq, Wk, Wv, Wo):
    global _CACHED_NC, _CACHED_NC_GENERIC

    in_maps, causal = _make_in_maps(
        hidden_states, attention_mask, position_ids, Wq, Wk, Wv, Wo
    )
    if causal:
        if _CACHED_NC is None:
            _CACHED_NC = build_nc(causal=True)
        nc = _CACHED_NC
    else:
        if _CACHED_NC_GENERIC is None:
            _CACHED_NC_GENERIC = build_nc(causal=False)
        nc = _CACHED_NC_GENERIC

    res = run_bass_kernel_spmd(nc, in_maps, core_ids=list(range(N_CORES)))

    out = np.zeros((B, S, H), dtype=np.float32)
    for c in range(N_CORES):
        b = c // GROUPS
        out[b] += res.results[c]["out"]
    return out


# revision 51
# speedup vs baseline: 1.0257x; 1.0257x over previous
"""LlamaAttention forward on 8 Trainium2 NeuronCores (Bass/Tile).

Sharding: 8 cores = 2 batches x 4 head-groups (4 heads each).
Per core: QKV projections (column-parallel over heads), RoPE, causal eager
attention for its 4 heads, and a row-parallel partial O-projection; the host
sums the 4 partials per batch.

Schedule (causal fast path):
  phase 1:  [V st0..7][Q h0][K h0][att span0 h0][V st8..11][Q h1][K h1]
            [att span0 h1][V 12..15][Q h2][K h2][att s0 h2][Q h3][K h3]
            [att s0 h3] -- span-0 attention (ACT-bound) hides behind the
            projection matmuls; RoPE chains are deferred one unit so the
            rot-matmul never stalls the PE behind the ACT psum-copy.
  phase 2:  spans 1..3; the O-projection of span s-1 is injected *inside*
            the attention kc-loops (4 x 4-matmul bundles per head) to keep
            the PE fed while ACT computes exp; remaining O tiles drain at
            the end with per-ospan output stores.

Everything is bf16 (same PE rate as fp32r, half the DMA/SBUF) with fp32
PSUM accumulation; Q/K/V stay SBUF-resident; blocks above the causal
diagonal are skipped, sub-diagonal blocks skip the mask multiply, and only
the diagonal band of exp(mask) is shipped.  Softmax denominators come from
a DVE bf16 accumulation + one gpsimd partition_all_reduce per head-span
(no ones-matmuls on the PE).  exp runs on unshifted scores (they are O(1);
the 1/sqrt(hd) scale is folded into the q-side RoPE tables).
"""

import math
import sys
from contextlib import ExitStack

import numpy as np

try:
    import concourse.bass as bass
except ImportError:  # pragma: no cover - fallback when PYTHONPATH is not set
    sys.path.insert(0, "/opt/trn_rl_repo")
    import concourse.bass as bass

import concourse.bacc as bacc
import concourse.tile as tile
from concourse import mybir
from concourse import bass_isa
from concourse.bass import ds, ts
from concourse.bass_utils import run_bass_kernel_spmd

import ml_dtypes

# Problem constants (hardcoded per spec)
B, S, H = 2, 2048, 2048
N_HEADS, HD = 16, 128
ROPE_BASE = 10000.0
N_CORES = 8
GROUPS = 4          # head-groups (tensor parallel)
HPG = N_HEADS // GROUPS  # heads per group = 4
D = HPG * HD        # per-core projected dim = 512
P = 128
HC = H // P         # contraction chunks for projections = 16
STILES = S // P     # s tiles = 16
NSPAN = 4           # 512-wide spans of S
SPAN = S // NSPAN   # 512
KC = S // P         # k chunks = 16

F32 = mybir.dt.float32
BF16 = mybir.dt.bfloat16
MUL = mybir.AluOpType.mult
ADD = mybir.AluOpType.add
EXP = mybir.ActivationFunctionType.Exp
COPY = mybir.ActivationFunctionType.Copy
RADD = bass_isa.ReduceOp.add

_CACHED_NC = None          # causal-structure kernel (fast path)
_CACHED_NC_GENERIC = None  # arbitrary-mask fallback


def build_nc(causal=True):
    nc = bacc.Bacc("TRN2", target_bir_lowering=False)

    # x pre-tiled on host: [p, st, hc, si] so both the V (st-chunk) and the
    # Q/K (span) access patterns stream contiguous >=512B runs.
    xt4 = nc.declare_dram_parameter("xt4", [P, STILES, HC, P], BF16, isOutput=False)
    wq4 = nc.declare_dram_parameter("wq4", [P, HPG, HC, HD], BF16, isOutput=False)
    wk4 = nc.declare_dram_parameter("wk4", [P, HPG, HC, HD], BF16, isOutput=False)
    wv2 = nc.declare_dram_parameter("wv2", [P, HC, D], BF16, isOutput=False)
    wo2 = nc.declare_dram_parameter("wo2", [P, HPG, H], BF16, isOutput=False)
    if causal:
        emd = nc.declare_dram_parameter("emd", [P, KC, SPAN], BF16, isOutput=False)
    else:
        emd = nc.declare_dram_parameter("emd", [P, KC, S], BF16, isOutput=False)
    # sinq/sink arrive sign-folded (rows 0..63 negated) so rotate_half is a
    # pure partition swap done by two SBUF->SBUF DMAs -- no PE matmul needed
    cosq = nc.declare_dram_parameter("cosq", [HD, S], BF16, isOutput=False)
    sinq = nc.declare_dram_parameter("sinq", [HD, S], BF16, isOutput=False)
    cosk = nc.declare_dram_parameter("cosk", [HD, S], BF16, isOutput=False)
    sink = nc.declare_dram_parameter("sink", [HD, S], BF16, isOutput=False)
    out = nc.declare_dram_parameter("out", [S, H], F32, isOutput=True)

    def n_kc(span):
        """number of k-chunks contributing to this q-span"""
        return 4 * span + 4 if causal else KC

    def needs_mask(span, kc):
        return kc >= 4 * span if causal else True

    with tile.TileContext(nc) as tc, ExitStack() as ctx_all:
        # long-lived tensors
        kv_pool = ctx_all.enter_context(tc.tile_pool(name="kv", bufs=1))
        qt_sb = kv_pool.tile([P, HPG, S], BF16)
        kt_sb = kv_pool.tile([P, HPG, S], BF16)
        v_sb = kv_pool.tile([P, KC, D], BF16)
        # one attnT tile+pool per head: coarse-grained RAW tracking would
        # otherwise stall O-projection matmuls on unrelated heads' normalizes
        attnT = []
        for h in range(HPG):
            p_h = ctx_all.enter_context(tc.tile_pool(name=f"attnT{h}", bufs=1))
            attnT.append(p_h.tile([P, S], BF16, name=f"attnT_{h}"))
        em_sb = None
        if causal:
            em_sb = kv_pool.tile([P, KC, SPAN], BF16, name="em_sb")

        # attention working pools (used from phase 1 on for span 0)
        et_pool = ctx_all.enter_context(tc.tile_pool(name="et", bufs=2))
        pt_pool = ctx_all.enter_context(tc.tile_pool(name="pt", bufs=2))
        acc_pool = ctx_all.enter_context(tc.tile_pool(name="acc", bufs=2))
        red_pool = ctx_all.enter_context(tc.tile_pool(name="red", bufs=1))
        rc_pool = ctx_all.enter_context(tc.tile_pool(name="rc", bufs=1))
        psum_at = ctx_all.enter_context(
            tc.tile_pool(name="psum_at", bufs=2, space="PSUM")
        )
        # shared space: holds wv during phase 1, recycled for wo afterwards
        wvwo_pool = ctx_all.enter_context(tc.tile_pool(name="wvwo", bufs=1))

        # O-projection task machinery (tasks injected into attention loops)
        o_state = {"queue": [], "emit": None}

        def pump_o():
            if o_state["queue"] and o_state["emit"] is not None:
                o_state["emit"](*o_state["queue"].pop(0))

        def attention_unit(span, head, inject, sc_pool, paired=False):
            nk = n_kc(span)
            marks = {((i + 1) * nk) // 5 for i in range(4)} if inject else ()
            qh = qt_sb[:, head, ts(span, SPAN)]
            aps = psum_at.tile([P, SPAN], F32, tag="atps")
            acc = acc_pool.tile([P, SPAN], BF16, tag="acc")
            step = 2 if paired else 1
            for kc0 in range(0, nk, step):
                kcs = range(kc0, kc0 + step)
                # scores for `step` k-blocks share one psum tile so exp's
                # per-instruction access latency is amortized
                sc = sc_pool.tile([P, step, SPAN], F32, tag="scps")
                for i, kc in enumerate(kcs):
                    nc.tensor.matmul(
                        sc[:, i],
                        lhsT=kt_sb[:, head, ts(kc, P)],
                        rhs=qh,
                        start=True,
                        stop=True,
                    )
                et = et_pool.tile([P, step, SPAN], BF16, tag="et")
                nc.scalar.activation(et[:], sc[:], EXP)
                if needs_mask(span, kc0):
                    if causal:
                        emb = em_sb[:, ds(kc0, step)]
                    else:
                        emb = em_sb[:, ds(kc0, step), ts(span, SPAN)]
                    pt = pt_pool.tile([P, step, SPAN], BF16, tag="pt")
                    nc.vector.tensor_tensor(pt[:], et[:], emb, MUL)
                    p_use = pt
                else:
                    p_use = et
                for i, kc in enumerate(kcs):
                    nc.tensor.matmul(
                        aps[:],
                        lhsT=v_sb[:, kc, ts(head, HD)],
                        rhs=p_use[:, i],
                        start=(kc == 0),
                        stop=(kc == nk - 1),
                    )
                    if kc == 0:
                        nc.vector.tensor_copy(acc[:], p_use[:, i])
                    else:
                        nc.vector.tensor_tensor(acc[:], acc[:], p_use[:, i], ADD)
                    if kc in marks:
                        pump_o()
            red = red_pool.tile([P, SPAN], F32, tag="red")
            nc.gpsimd.partition_all_reduce(
                red[:], acc[:], channels=P, reduce_op=RADD
            )
            rcb = rc_pool.tile([P, SPAN], F32, tag="rcb")
            nc.vector.reciprocal(rcb[:], red[:])
            nc.vector.tensor_tensor(
                attnT[head][:, ts(span, SPAN)], aps[:], rcb[:], MUL
            )

        # ================= Phase 1: projections + RoPE (+ span-0 attn) =====
        with ExitStack() as ctx_p1:
            xt_pool = ctx_p1.enter_context(tc.tile_pool(name="xt", bufs=1))
            psum_v = ctx_p1.enter_context(
                tc.tile_pool(name="psum_v", bufs=2, space="PSUM")
            )
            tab_pool = ctx_p1.enter_context(tc.tile_pool(name="tab", bufs=1))
            wp_pool = ctx_p1.enter_context(tc.tile_pool(name="wp", bufs=2))
            rawp = ctx_p1.enter_context(tc.tile_pool(name="rawp", bufs=2))
            rswp = ctx_p1.enter_context(tc.tile_pool(name="rswp", bufs=2))
            t1p = ctx_p1.enter_context(tc.tile_pool(name="t1p", bufs=2))
            t2p = ctx_p1.enter_context(tc.tile_pool(name="t2p", bufs=2))
            psum_sc = ctx_p1.enter_context(
                tc.tile_pool(name="psum_sc", bufs=4, space="PSUM")
            )

            xt_sb = xt_pool.tile([P, STILES, HC, P], BF16)
            wv_sb = wvwo_pool.tile([P, HC, D], BF16, tag="wvwo", name="wv_sb")

            # input DMAs, interleaved so V st0 can start ~3us in
            for g in range(4):
                nc.sync.dma_start(xt_sb[:, 0, ts(g, 4)], xt4[:, 0, ts(g, 4)])
                nc.sync.dma_start(wv_sb[:, ts(g, 4)], wv2[:, ts(g, 4)])
            for st in range(1, 8):
                nc.sync.dma_start(xt_sb[:, st], xt4[:, st])
            cq = tab_pool.tile([HD, S], BF16)
            sq = tab_pool.tile([HD, S], BF16)
            ck = tab_pool.tile([HD, S], BF16)
            sk = tab_pool.tile([HD, S], BF16)
            nc.sync.dma_start(cq[:], cosq[:])
            nc.sync.dma_start(sq[:], sinq[:])
            for st in range(8, STILES):
                nc.sync.dma_start(xt_sb[:, st], xt4[:, st])
            nc.sync.dma_start(ck[:], cosk[:])
            nc.sync.dma_start(sk[:], sink[:])
            if causal:
                nc.sync.dma_start(em_sb[:], emd[:])

            def vunit(st):
                ps = psum_v.tile([P, D], F32, tag="vps")
                for hc in range(HC):
                    nc.tensor.matmul(
                        ps[:],
                        lhsT=xt_sb[:, st, hc],
                        rhs=wv_sb[:, hc],
                        start=(hc == 0),
                        stop=(hc == HC - 1),
                    )
                nc.scalar.activation(v_sb[:, st], ps[:], COPY)

            # deferred rope: (psum, cos_ap, sin_ap, dest_ap)
            pending = [None]

            def emit_rope():
                ps, tabc, tabs, dest_ap = pending[0]
                pending[0] = None
                raw = rawp.tile([P, SPAN], BF16, tag="raw")
                nc.scalar.activation(raw[:], ps[:], COPY)
                # rotate_half = partition-swap (sign lives in the sin tables)
                rsw = rswp.tile([P, SPAN], BF16, tag="rsw")
                nc.sync.dma_start(rsw[0 : HD // 2], raw[HD // 2 : HD])
                nc.sync.dma_start(rsw[HD // 2 : HD], raw[0 : HD // 2])
                t1 = t1p.tile([P, SPAN], BF16, tag="t1")
                nc.vector.tensor_tensor(t1[:], raw[:], tabc, MUL)
                t2 = t2p.tile([P, SPAN], BF16, tag="t2")
                nc.vector.tensor_tensor(t2[:], rsw[:], tabs, MUL)
                nc.vector.tensor_tensor(dest_ap, t1[:], t2[:], ADD)

            def qkblock(head, w4, tabc, tabs, dest, mid_cb=None):
                wp = wp_pool.tile([P, HC, HD], BF16, tag="wp")
                nc.sync.dma_start(wp[:], w4[:, head])
                for span in range(NSPAN):
                    ps = psum_sc.tile([P, SPAN], F32, tag="scps")
                    for hc in range(HC):
                        nc.tensor.matmul(
                            ps[:],
                            lhsT=wp[:, hc],
                            rhs=xt_sb[:, ts(span, 4), hc, :],
                            start=(hc == 0),
                            stop=(hc == HC - 1),
                        )
                    if pending[0] is not None:
                        emit_rope()
                    pending[0] = (
                        ps,
                        tabc[:, ts(span, SPAN)],
                        tabs[:, ts(span, SPAN)],
                        dest[:, head, ts(span, SPAN)],
                    )
                    if span == 2 and mid_cb is not None:
                        # span-0 rope finished during span 1/2: the callback's
                        # attention unit overlaps the span-3 matmuls
                        mid_cb()

            v_rounds = [range(0, 8), range(8, 12), range(12, 16), range(0, 0)]
            wo_sb = None
            for head in range(HPG):
                for st in v_rounds[head]:
                    vunit(st)
                if head == HPG - 1:
                    # all V units emitted: recycle wv's space for wo so
                    # span-1's O-projection never waits on its load
                    wo_sb = wvwo_pool.tile(
                        [P, HPG, H], BF16, tag="wvwo", name="wo_sb"
                    )
                    nc.sync.dma_start(wo_sb[:], wo2[:])
                qkblock(head, wq4, cq, sq, qt_sb)
                att0 = (
                    (
                        lambda h: lambda: attention_unit(
                            0, h, inject=False, sc_pool=psum_sc
                        )
                    )(head)
                    if causal
                    else None
                )
                if head == HPG - 1:
                    # last head: run span-0 attention inside the K block so
                    # its ACT chain hides behind the span-3 matmuls
                    qkblock(head, wk4, ck, sk, kt_sb, mid_cb=att0)
                else:
                    qkblock(head, wk4, ck, sk, kt_sb)
                    if att0 is not None:
                        att0()
            if pending[0] is not None:
                emit_rope()

        # ================= Phase 2 + 3 =================
        with ExitStack() as ctx_p23:
            psum_sc2 = ctx_p23.enter_context(
                tc.tile_pool(name="psum_sc2", bufs=2, space="PSUM")
            )
            if not causal:
                emf_pool = ctx_p23.enter_context(tc.tile_pool(name="emf", bufs=1))
                em_sb = emf_pool.tile([P, KC, S], BF16, name="em_sb")
                for g in range(4):
                    nc.sync.dma_start(em_sb[:, ts(g, 4)], emd[:, ts(g, 4)])
            o_pool = ctx_p23.enter_context(tc.tile_pool(name="o", bufs=6))
            psum_o = ctx_p23.enter_context(
                tc.tile_pool(name="psum_o", bufs=2, space="PSUM")
            )

            o_flip = [0]

            def emit_o_mms(ps, st, ospan, dcs, start, stop):
                for i, dc in enumerate(dcs):
                    nc.tensor.matmul(
                        ps[:],
                        lhsT=attnT[dc][:, ts(st, P)],
                        rhs=wo_sb[:, dc, ts(ospan, SPAN)],
                        start=start and (i == 0),
                        stop=stop and (i == len(dcs) - 1),
                    )

            def emit_o_store(ps, st, ospan):
                o_sb = o_pool.tile([P, SPAN], F32, tag="o")
                if o_flip[0] % 2 == 0:
                    nc.vector.tensor_copy(o_sb[:], ps[:])
                else:
                    nc.scalar.activation(o_sb[:], ps[:], COPY)
                o_flip[0] += 1
                nc.sync.dma_start(out[ts(st, P), ts(ospan, SPAN)], o_sb[:])

            o_pools = [psum_o, psum_at]

            def emit_o(st, ospan):
                pool = o_pools[o_flip[0] % len(o_pools)]
                ps = pool.tile(
                    [P, SPAN], F32, tag="ops" if pool is psum_o else "atps"
                )
                emit_o_mms(ps, st, ospan, range(HPG), True, True)
                emit_o_store(ps, st, ospan)

            o_state["emit"] = emit_o

            first_span = 1 if causal else 0
            if causal:
                o_state["queue"] = [
                    (st, osp) for st in range(4) for osp in range(NSPAN)
                ]
            for span in range(first_span, NSPAN):
                for head in range(HPG):
                    attention_unit(
                        span, head, inject=True, sc_pool=psum_sc2, paired=True
                    )
                o_state["queue"].extend(
                    (4 * span + i, osp) for i in range(4) for osp in range(NSPAN)
                )
            # drain prologue: open 4 tiles with their dc 0..2 accumulated
            # (independent of the last head's normalize), then finish
            pre = []
            for _ in range(min(4, len(o_state["queue"]))):
                st, osp = o_state["queue"].pop(0)
                pool = o_pools[o_flip[0] % len(o_pools)]
                ps = pool.tile(
                    [P, SPAN], F32, tag="ops" if pool is psum_o else "atps"
                )
                o_flip[0] += 1
                emit_o_mms(ps, st, osp, range(HPG - 1), True, False)
                pre.append((ps, st, osp))
            for ps, st, osp in pre:
                emit_o_mms(ps, st, osp, [HPG - 1], False, True)
                emit_o_store(ps, st, osp)
            while o_state["queue"]:
                pump_o()

    nc.compile()
    return nc


def _rope_tables(position_ids_b):
    """cos/sin tables [S, HD], matching the reference computation."""
    inv_freq = (
        1.0 / (ROPE_BASE ** (np.arange(0, HD, 2, dtype=np.float32) / HD))
    ).astype(np.float32)
    t = np.arange(S, dtype=np.float32)
    freqs = np.outer(t, inv_freq).astype(np.float32)  # [S, HD/2]
    emb = np.concatenate([freqs, freqs], axis=-1)  # [S, HD]
    cos = np.cos(emb).astype(np.float32)
    sin = np.sin(emb).astype(np.float32)
    pos = np.asarray(position_ids_b).astype(np.int64)
    return cos[pos], sin[pos]  # [S, HD] each


def _causal_band(em):
    """Extract the diagonal band [p, 4j+c, q] of exp(mask).T blocks."""
    band = np.empty((P, KC, SPAN), dtype=ml_dtypes.bfloat16)
    for j in range(NSPAN):
        sub = em[512 * j : 512 * j + 512, 512 * j : 512 * j + 512]
        band[:, 4 * j : 4 * j + 4, :] = sub.reshape(4, 128, SPAN).transpose(1, 0, 2)
    return band


def _is_causal_structured(em):
    """True if blocks left of the diagonal band are all-1 and blocks right
    of it are all-0 (band content itself is shipped verbatim)."""
    for j in range(NSPAN):
        qs = slice(512 * j, 512 * j + 512)
        if not np.all(em[: 512 * j, qs] == 1.0):
            return False
        if not np.all(em[512 * j + 512 :, qs] == 0.0):
            return False
    return True


def _make_in_maps(hidden_states, attention_mask, position_ids, Wq, Wk, Wv, Wo):
    hidden_states = np.asarray(hidden_states, dtype=np.float32)
    attention_mask = np.asarray(attention_mask, dtype=np.float32)
    Wq = np.asarray(Wq, dtype=np.float32)
    Wk = np.asarray(Wk, dtype=np.float32)
    Wv = np.asarray(Wv, dtype=np.float32)
    Wo = np.asarray(Wo, dtype=np.float32)
    bf = ml_dtypes.bfloat16

    # rotate_half on-device is a partition swap; the sign of the -x2 half is
    # folded into the sin tables (rows 0..63 negated)
    sin_sign = np.ones((HD, 1), dtype=np.float32)
    sin_sign[: HD // 2] = -1.0

    scale = np.float32(1.0 / np.sqrt(HD))

    # per-group weight tensors (shared across batches)
    gw = []
    for g in range(GROUPS):
        dsl = slice(g * D, (g + 1) * D)
        wq_g = np.empty((P, HPG, HC, HD), dtype=bf)
        wk_g = np.empty((P, HPG, HC, HD), dtype=bf)
        for h in range(HPG):
            rows = slice(g * D + h * HD, g * D + h * HD + HD)
            # W rows [HD, H] -> transpose [H, HD] -> [hc, p, hd] -> [p, hc, hd]
            wq_g[:, h] = (
                Wq[rows, :].T.reshape(HC, P, HD).transpose(1, 0, 2).astype(bf)
            )
            wk_g[:, h] = (
                Wk[rows, :].T.reshape(HC, P, HD).transpose(1, 0, 2).astype(bf)
            )
        wv_g = (
            Wv[dsl, :].T.reshape(HC, P, D).transpose(1, 0, 2).astype(bf)
        )  # [p, hc, d]
        wo_g = (
            Wo[:, dsl].T.reshape(HPG, P, H).transpose(1, 0, 2).astype(bf)
        )  # [p, dc, o]
        gw.append((wq_g, wk_g, wv_g, wo_g))

    causal_all = True
    per_batch = []
    for b in range(B):
        xT = hidden_states[b].T  # [H, S]
        x4 = (
            xT.reshape(HC, P, STILES, P).transpose(1, 2, 0, 3).astype(bf)
        )  # [p, st, hc, si]
        mask_b = attention_mask[b, 0]  # [S, S] additive
        with np.errstate(over="ignore", under="ignore"):
            em = np.exp(mask_b.T, dtype=np.float32)  # [k, q]
        causal_b = _is_causal_structured(em)
        causal_all = causal_all and causal_b
        cos_b, sin_b = _rope_tables(position_ids[b])  # [S, HD]
        cosT = cos_b.T  # [HD, S]
        sinT = sin_b.T * sin_sign
        tables = {
            "cosq": (cosT * scale).astype(bf),
            "sinq": (sinT * scale).astype(bf),
            "cosk": cosT.astype(bf),
            "sink": sinT.astype(bf),
        }
        per_batch.append((x4, em, tables))

    in_maps = []
    for b in range(B):
        x4, em, tables = per_batch[b]
        if causal_all:
            em_t = _causal_band(em)
        else:
            em_t = (
                em.astype(bf).reshape(KC, P, S).transpose(1, 0, 2)
            )  # [p, kc, q]
        for g in range(GROUPS):
            wq_g, wk_g, wv_g, wo_g = gw[g]
            in_maps.append(
                {
                    "xt4": x4,
                    "wq4": wq_g,
                    "wk4": wk_g,
                    "wv2": wv_g,
                    "wo2": wo_g,
                    "emd": np.ascontiguousarray(em_t),
                    **tables,
                }
            )
    return in_maps, causal_all


def kernel(hidden_states, attention_mask, position_ids, Wq, Wk, Wv, Wo):
    global _CACHED_NC, _CACHED_NC_GENERIC

    in_maps, causal = _make_in_maps(
        hidden_states, attention_mask, position_ids, Wq, Wk, Wv, Wo
    )
    if causal:
        if _CACHED_NC is None:
            _CACHED_NC = build_nc(causal=True)
        nc = _CACHED_NC
    else:
        if _CACHED_NC_GENERIC is None:
            _CACHED_NC_GENERIC = build_nc(causal=False)
        nc = _CACHED_NC_GENERIC

    res = run_bass_kernel_spmd(nc, in_maps, core_ids=list(range(N_CORES)))

    out = np.zeros((B, S, H), dtype=np.float32)
    for c in range(N_CORES):
        b = c // GROUPS
        out[b] += res.results[c]["out"]
    return out
